# revision 15
# baseline (speedup 1.0000x reference)
"""Trainium2 Bass kernel for nn_AttLayer (sparse window attention).

Self-contained: accepts FULL inputs, shards time dim across 8 NeuronCores
(each core: 4 batches x 4 attention blocks with a 128-col halo), runs one
SPMD NEFF, gathers the full output on the host.
"""

import sys

for _p in ("/opt/trn_rl_repo",):
    if _p not in sys.path:
        sys.path.insert(0, _p)

import numpy as np

# Problem constants (hardcoded per spec)
_B, _C, _T = 4, 512, 8192
_E = 256          # embed dim after compression
_D = 256          # dilation / block size (queries per block)
_P = 128          # window pad (d // 2)
_W = 512          # window length used in attention (d + 2p)
_NC = 8           # cores
_TL = _T // _NC   # local columns per core (1024)
_TH = _TL + 2 * _P  # halo-padded local columns (1280)
_NBL = _TL // _D  # local blocks per core (4)
_KC = _C // 128   # K-chunks over input channels (4)
_EC = _E // 128   # chunks over E (2)
_O = 512          # output channels
_OC = _O // 128   # chunks over O (4)
_CC = _TH // 128  # col-chunks for vT (10)

_cached = {}


def _build():
    import concourse.bass as bass
    import concourse.mybir as mybir
    import concourse.tile as tile
    from concourse import bacc
    import contextlib

    f32 = mybir.dt.float32
    f32r = mybir.dt.float32r
    AF = mybir.ActivationFunctionType

    nc = bacc.Bacc("TRN2", target_bir_lowering=False, debug=False, num_devices=_NC)

    x_dr = nc.declare_dram_parameter("x", [_B, _C, _TH], f32r, isOutput=False)
    wq_dr = nc.declare_dram_parameter("wq", [_C, _E], f32r, isOutput=False)
    wk_dr = nc.declare_dram_parameter("wk", [_C, _E], f32r, isOutput=False)
    wv_dr = nc.declare_dram_parameter("wv", [_C, _E], f32r, isOutput=False)
    wo_dr = nc.declare_dram_parameter("wo", [_E, _O], f32r, isOutput=False)
    bq_dr = nc.declare_dram_parameter("bq", [128, _EC], f32, isOutput=False)
    bk_dr = nc.declare_dram_parameter("bk", [128, _EC], f32, isOutput=False)
    bvt_dr = nc.declare_dram_parameter("bvt", [128, _E], f32, isOutput=False)
    bo_dr = nc.declare_dram_parameter("bo", [128, _OC], f32, isOutput=False)
    logm_dr = nc.declare_dram_parameter("logm", [128, _B * _NBL * 4], f32, isOutput=False)
    ones_dr = nc.declare_dram_parameter("ones128", [128, 128], f32r, isOutput=False)
    out_dr = nc.declare_dram_parameter("out", [_B, _O, _TL], f32, isOutput=True)

    with tile.TileContext(nc) as tc:
        with contextlib.ExitStack() as ctx:
            consts = ctx.enter_context(tc.tile_pool(name="consts", bufs=1))
            xin = ctx.enter_context(tc.tile_pool(name="xin", bufs=8))
            qkp = ctx.enter_context(tc.tile_pool(name="qk", bufs=2))
            vtp = ctx.enter_context(tc.tile_pool(name="vt", bufs=2))
            wxp = ctx.enter_context(tc.tile_pool(name="wx", bufs=3))
            smallp = ctx.enter_context(tc.tile_pool(name="small", bufs=2))
            ps = ctx.enter_context(
                tc.tile_pool(name="ps", bufs=6, space=bass.MemorySpace.PSUM)
            )
            avdb = ctx.enter_context(
                tc.tile_pool(name="avdb", bufs=2, space=bass.MemorySpace.PSUM)
            )

            # --- constants ---
            wq_sb = consts.tile([128, _KC, _E], f32r, tag="wq")
            nc.sync.dma_start(wq_sb[:], wq_dr.ap().rearrange("(k p) e -> p k e", p=128))
            wk_sb = consts.tile([128, _KC, _E], f32r, tag="wk")
            nc.sync.dma_start(wk_sb[:], wk_dr.ap().rearrange("(k p) e -> p k e", p=128))
            wv_sb = consts.tile([128, _KC, _E], f32r, tag="wv")
            nc.sync.dma_start(wv_sb[:], wv_dr.ap().rearrange("(k p) e -> p k e", p=128))
            wo_sb = consts.tile([128, _EC, _O], f32r, tag="wo")
            nc.sync.dma_start(wo_sb[:], wo_dr.ap().rearrange("(k p) o -> p k o", p=128))
            bq_sb = consts.tile([128, _EC], f32, tag="bq")
            nc.sync.dma_start(bq_sb[:], bq_dr.ap())
            bk_sb = consts.tile([128, _EC], f32, tag="bk")
            nc.sync.dma_start(bk_sb[:], bk_dr.ap())
            bvt_sb = consts.tile([128, _E], f32, tag="bvt")
            nc.sync.dma_start(bvt_sb[:], bvt_dr.ap())
            bo_sb = consts.tile([128, _OC], f32, tag="bo")
            nc.sync.dma_start(bo_sb[:], bo_dr.ap())
            logm_sb = consts.tile([128, _B * _NBL * 4], f32, tag="logm")
            nc.sync.dma_start(logm_sb[:], logm_dr.ap())
            ones_sb = consts.tile([128, 128], f32r, tag="ones")
            nc.sync.dma_start(ones_sb[:], ones_dr.ap())

            for b in range(_B):
                # --- load input K-chunks ---
                xts = []
                for kc in range(_KC):
                    xt = xin.tile([128, _TH], f32r, tag="x")
                    nc.sync.dma_start(
                        xt[:],
                        x_dr.ap()[b].rearrange("(k p) t -> k p t", p=128)[kc],
                    )
                    xts.append(xt)

                # --- q, k projections: [E(part), t] layout; bias add on DVE ---
                q_sb = qkp.tile([128, _EC, _TL], f32r, tag="q")
                k_sb = qkp.tile([128, _EC, _TH], f32r, tag="k")
                with nc.allow_low_precision(reason="f32r is 32-bit storage"):
                    for w_sb_, bias_sb, dst, ncols, t0, use_act in (
                        (wq_sb, bq_sb, q_sb, _TL, _P, False),
                        (wk_sb, bk_sb, k_sb, _TH, 0, True),
                    ):
                        nchunks = (ncols + 511) // 512
                        for mc in range(_EC):
                            for ncix in range(nchunks):
                                n0 = ncix * 512
                                nn = min(512, ncols - n0)
                                pt = ps.tile([128, 512], f32, tag="ps")
                                for kc in range(_KC):
                                    nc.tensor.matmul(
                                        pt[:, :nn],
                                        w_sb_[:, kc, mc * 128 : (mc + 1) * 128],
                                        xts[kc][:, t0 + n0 : t0 + n0 + nn],
                                        start=(kc == 0),
                                        stop=(kc == _KC - 1),
                                    )
                                if use_act:
                                    nc.scalar.activation(
                                        dst[:, mc, n0 : n0 + nn],
                                        pt[:, :nn],
                                        AF.Identity,
                                        bias=bias_sb[:, mc : mc + 1],
                                    )
                                else:
                                    nc.vector.tensor_scalar_add(
                                        dst[:, mc, n0 : n0 + nn],
                                        pt[:, :nn],
                                        bias_sb[:, mc : mc + 1],
                                    )

                    # --- v projection, transposed: vT [t(part), E] ---
                    vt_sb = vtp.tile([128, _CC, _E], f32r, tag="vt")
                    for cc in range(_CC):
                        pt = ps.tile([128, _E], f32, tag="ps")
                        for kc in range(_KC):
                            nc.tensor.matmul(
                                pt[:],
                                xts[kc][:, cc * 128 : (cc + 1) * 128],
                                wv_sb[:, kc, :],
                                start=(kc == 0),
                                stop=(kc == _KC - 1),
                            )
                        nc.vector.tensor_add(vt_sb[:, cc, :], pt[:], bvt_sb[:])

                # --- attention blocks: software-pipelined so block n+1's
                # energy matmuls keep the PE busy while ACT runs block n's
                # exp; then batch-grouped gelu, then proj2 ---
                def emit_energy(nl):
                    col0 = nl * _D  # window start in halo coords
                    e_tiles = [
                        ps.tile([128, 512], f32, tag="ps", name=f"e_{b}_{nl}_{i}")
                        for i in range(2)
                    ]
                    for wc in range(4):
                        et = e_tiles[wc // 2]
                        eoff = (wc % 2) * 256
                        for ec in range(_EC):
                            nc.tensor.matmul(
                                et[:, eoff : eoff + 256],
                                k_sb[:, ec, col0 + wc * 128 : col0 + (wc + 1) * 128],
                                q_sb[:, ec, nl * _D : (nl + 1) * _D],
                                start=(ec == 0),
                                stop=(ec == _EC - 1),
                            )
                    # w = exp(e/16 + log(mask+1e-6)); mask bias per partition
                    wx = wxp.tile([128, 4, 256], f32r, tag="w", name=f"wx_{b}_{nl}")
                    for wc in range(4):
                        nc.scalar.activation(
                            wx[:, wc, :],
                            e_tiles[wc // 2][:, (wc % 2) * 256 : (wc % 2) * 256 + 256],
                            AF.Exp,
                            scale=1.0 / 16.0,
                            bias=logm_sb[
                                :, b * 16 + nl * 4 + wc : b * 16 + nl * 4 + wc + 1
                            ],
                        )
                    return wx

                t_tiles = [
                    smallp.tile(
                        [128, _EC, 2, 256], f32, tag="t", name=f"t_{b}_{h}", bufs=3
                    )
                    for h in range(_NBL // 2)
                ]
                wx_next = emit_energy(0)
                for nl in range(_NBL):
                    wx = wx_next
                    if nl + 1 < _NBL:
                        wx_next = emit_energy(nl + 1)

                    # AV: av[e,q] = sum_j v[e,j] w[j,q]  (contiguous groups:
                    # start=True clears has_written for the whole PSUM bank).
                    # Last chunk contracts K=127: drops window col 511,
                    # which the reference's window mask excludes.
                    av = avdb.tile([128, 512], f32, tag="av", name=f"av_{b}_{nl}")
                    for mg in range(_EC):
                        for wc in range(4):
                            kk = 127 if wc == 3 else 128
                            nc.tensor.matmul(
                                av[:, mg * 256 : (mg + 1) * 256],
                                vt_sb[:kk, 2 * nl + wc, mg * 128 : (mg + 1) * 128],
                                wx[:kk, wc, :],
                                start=(wc == 0),
                                stop=(wc == 3),
                            )
                    # Db[p,q] = sum_j w[j,q] broadcast to all partitions
                    db = avdb.tile([128, 256], f32, tag="av", name=f"db_{b}_{nl}")
                    for wc in range(4):
                        kk = 127 if wc == 3 else 128
                        nc.tensor.matmul(
                            db[:],
                            ones_sb[:kk, :],
                            wx[:kk, wc, :],
                            start=(wc == 0),
                            stop=(wc == 3),
                        )
                    rb_sb = smallp.tile([128, 256], f32, tag="rb", bufs=3)
                    nc.vector.reciprocal_approx_fast(rb_sb[:], db[:])
                    for mg in range(_EC):
                        nc.vector.tensor_mul(
                            t_tiles[nl // 2][:, mg, nl % 2, :],
                            av[:, mg * 256 : (mg + 1) * 256],
                            rb_sb[:],
                        )

                # gelu for all blocks of the batch (exact erf-based),
                # grouped so ACT swaps Exp<->Gelu tables once per batch
                g_tiles = []
                for half in range(_NBL // 2):
                    g_sb = smallp.tile(
                        [128, _EC, 2, 256], f32r, tag="g", name=f"g_{b}_{half}"
                    )
                    g_tiles.append(g_sb)
                    for mg in range(_EC):
                        nc.scalar.activation(
                            g_sb[:, mg, :, :], t_tiles[half][:, mg, :, :], AF.Gelu
                        )

                # output projection per pair: N=512
                for half in range(_NBL // 2):
                    g_sb = g_tiles[half]
                    o_tiles = [
                        ps.tile([128, 512], f32, tag="ps", name=f"o_{b}_{half}_{i}")
                        for i in range(_OC)
                    ]
                    for og in range(_OC):
                        for kc2 in range(_EC):
                            nc.tensor.matmul(
                                o_tiles[og][:],
                                wo_sb[:, kc2, og * 128 : (og + 1) * 128],
                                g_sb[:, kc2, :, :],
                                start=(kc2 == 0),
                                stop=(kc2 == _EC - 1),
                            )
                    out_sb = smallp.tile([128, _OC, 512], f32, tag="o")
                    for og in range(_OC):
                        nc.vector.tensor_scalar_add(
                            out_sb[:, og, :],
                            o_tiles[og][:],
                            bo_sb[:, og : og + 1],
                        )
                    nc.sync.dma_start(
                        out_dr.ap()[b].rearrange("(m p) t -> p m t", p=128)[
                            :, :, half * 512 : (half + 1) * 512
                        ],
                        out_sb[:],
                    )

    nc.compile()
    return nc


def _host_prep(inputs):
    """Shard full inputs into per-core in_maps."""
    x = np.asarray(inputs["input"], np.float32)
    mask = np.asarray(inputs["mask"], np.float32)
    Wq = np.asarray(inputs["Wq"], np.float32)
    bq = np.asarray(inputs["bq"], np.float32)
    Wk = np.asarray(inputs["Wk"], np.float32)
    bk = np.asarray(inputs["bk"], np.float32)
    Wv = np.asarray(inputs["Wv"], np.float32)
    bv = np.asarray(inputs["bv"], np.float32)
    Wo = np.asarray(inputs["Wo"], np.float32)
    bo = np.asarray(inputs["bo"], np.float32)

    wqT = np.ascontiguousarray(Wq.T)  # [C, E]
    wkT = np.ascontiguousarray(Wk.T)
    wvT = np.ascontiguousarray(Wv.T)
    woT = np.ascontiguousarray(Wo.T)  # [E, O]
    bq_dev = np.ascontiguousarray(bq.reshape(_EC, 128).T)
    bk_dev = np.ascontiguousarray(bk.reshape(_EC, 128).T)
    bo_dev = np.ascontiguousarray(bo.reshape(_OC, 128).T)
    bvt_dev = np.ascontiguousarray(np.broadcast_to(bv[None, :], (128, _E)))

    # window masks (exactly the reference's final_mask), per (b, global block)
    nb = _T // _D
    pm = mask[:, 0, :]  # (B, T); no tail padding needed since T % d == 0
    mp = np.pad(pm, ((0, 0), (_P, _P)))
    idx = np.arange(nb)[:, None] * _D + np.arange(_W)[None, :]
    mw = mp[:, idx]  # (B, nb, W)
    # win mask (col 511) handled on-device by K=127 contraction
    logm = np.log(mw + 1e-6).astype(np.float32)  # (B, nb, W)

    in_maps = []
    for c in range(_NC):
        base = c * _TL
        xs = np.zeros((_B, _C, _TH), np.float32)
        lo = base - _P
        hi = base + _TL + _P
        glo, ghi = max(lo, 0), min(hi, _T)
        xs[:, :, glo - lo : ghi - lo] = x[:, :, glo:ghi]

        # logm_dev[p, b*16 + nl*4 + wc] = logm[b, c*4+nl, wc*128+p]
        lm = logm[:, c * _NBL : (c + 1) * _NBL, :].reshape(_B, _NBL, 4, 128)
        logm_dev = np.ascontiguousarray(lm.transpose(3, 0, 1, 2).reshape(128, -1))

        in_maps.append(
            {
                "x": xs,
                "wq": wqT,
                "wk": wkT,
                "wv": wvT,
                "wo": woT,
                "bq": bq_dev,
                "bk": bk_dev,
                "bvt": bvt_dev,
                "bo": bo_dev,
                "logm": logm_dev,
                "ones128": np.ones((128, 128), np.float32),
            }
        )
    return in_maps, mask


def _run(inputs, trace=False):
    from concourse.bass_utils import run_bass_kernel_spmd

    if "nc" not in _cached:
        _cached["nc"] = _build()
    nc = _cached["nc"]
    in_maps, mask = _host_prep(inputs)
    res = run_bass_kernel_spmd(nc, in_maps, core_ids=list(range(_NC)), trace=trace)
    out = np.concatenate([res.results[c]["out"] for c in range(_NC)], axis=2)
    out = out * mask[:, 0:1, :]
    return out.astype(np.float32), res


def kernel(**inputs):
    out, _ = _run(inputs, trace=False)
    return out
